# revision 1
# baseline (speedup 1.0000x reference)
"""Trainium2 Bass kernel for nn_CudaMixedBitLinear (GPTQ-style 4-bit linear).

out[b,s,o] = sum_k x[b,s,k] * W[o,k],  W[o,k] = (q[o,k] - z[o,g]) * s[o,g],
g = k // 128, q/z packed as nibbles (low nibble first) in int32 bytes.

Sharding: column-parallel over out_features across 8 cores (11008 -> 1376
per core), x replicated (host-transposed to x^T for [k, m] tile loads),
outputs concatenated on host. No collectives.

Per-core kernel:
  Phase A (once): load packed qweight slice, unpack nibbles into fp16 via
    bit tricks on DVE (mantissa-OR: 0x6400|n == fp16(1024+n); the 1024 bias
    folds into the zero-point term), dequantize per 128-col group with
    dual-op tensor_scalar, PE-transpose 128x128 blocks into an SBUF-resident
    W^T laid out [k_partition, o] per k-tile (32 tiles x [128, 1376] fp16).
  Phase B: for each 256-col m-block of x^T, SWDGE-load [128, KT, 256] x^T
    tiles, then per 128-row m-tile accumulate over 32 k-tiles into 3 PSUM
    column chunks (512/512/352) with fp16 matmuls, copy PSUM->SBUF on ACT,
    DMA out via SWDGE.

All DMAs ride SWDGE (gpsimd): HWDGE descriptors allow only one sync wait,
which Tile's dependency waits can exceed.
"""

import numpy as np

B, S, K = 2, 2048, 4096
OUT_F = 11008
N_CORES = 8
OC = OUT_F // N_CORES       # 1376 out features per core
GROUP = 128
GROUPS = K // GROUP         # 32
M = B * S                   # 4096 rows
KT = K // 128               # 32 k-tiles
OT = (OC + 127) // 128      # 11 o-tiles (last has 96 rows)
CHUNKS = [(0, 512), (512, 1024), (1024, OC)]
XB = 256                    # m columns per x^T block buffer
NB = M // XB                # 16 m-blocks
SUBS = XB // 128            # 2 m-tiles per block

_CACHE = {}
RUN_KWARGS = {}   # test harness can inject e.g. dict(trace=True)
LAST_RESULT = None


def _build_bass():
    import concourse.bass as bass
    import concourse.bacc as bacc
    import concourse.mybir as mybir
    from concourse.tile import TileContext
    from concourse.masks import make_identity

    A = mybir.AluOpType
    fp16 = mybir.dt.float16
    f32 = mybir.dt.float32
    i32 = mybir.dt.int32

    nc = bacc.Bacc("TRN2", target_bir_lowering=False)
    xT = nc.dram_tensor("xt_dram", [K, M], fp16, kind="ExternalInput")
    qw = nc.dram_tensor("qweight", [OC, K // 2], i32, kind="ExternalInput")
    sc = nc.dram_tensor("scales", [OC, GROUPS], fp16, kind="ExternalInput")
    qz = nc.dram_tensor("qzeros", [OC, GROUPS // 2], i32, kind="ExternalInput")
    out = nc.dram_tensor("out", [M, OC], f32, kind="ExternalOutput")

    # static SBUF (never address-shared): resident W^T, x^T ring, out ring
    wt = nc.alloc_sbuf_tensor("wt", [128, KT * OC], fp16).ap()
    xts = [nc.alloc_sbuf_tensor(f"xtbuf{i}", [128, KT, XB], fp16).ap()
           for i in range(2)]
    obs = [nc.alloc_sbuf_tensor(f"obbuf{i}", [128, OC], f32).ap()
           for i in range(2)]
    ident = nc.alloc_sbuf_tensor("ident", [128, 128], fp16).ap()

    xT_view = xT[:, :].rearrange("(kt p) m -> p kt m", p=128)  # [128, KT, M]

    with TileContext(nc) as tc:
        with (
            tc.tile_pool(name="deq", bufs=2) as deq,
            tc.tile_pool(name="psA", bufs=2, space="PSUM") as ppA,
            tc.tile_pool(name="psB", bufs=2, space="PSUM") as ppB,
        ):
            make_identity(nc, ident)

            # ---------------- Phase A: dequantize ----------------
            def emit_otile(t):
                o0 = t * 128
                osz = min(128, OC - o0)
                q_t = deq.tile([128, K // 2], i32, tag="q", name="q_t")
                nc.gpsimd.dma_start(out=q_t[:osz], in_=qw[o0:o0 + osz, :])
                s_t = deq.tile([128, GROUPS], fp16, tag="s", name="s_t")
                nc.gpsimd.dma_start(out=s_t[:osz], in_=sc[o0:o0 + osz, :])
                z_t = deq.tile([128, GROUPS // 2], i32, tag="z", name="z_t")
                nc.gpsimd.dma_start(out=z_t[:osz], in_=qz[o0:o0 + osz, :])

                # zeros -> fp16(1024+z) via mantissa-OR trick
                z1 = deq.tile([128, GROUPS // 2], i32, tag="z1", name="z1")
                z2 = deq.tile([128, GROUPS // 2], i32, tag="z2", name="z2")
                nc.vector.tensor_scalar(out=z1[:osz], in0=z_t[:osz], scalar1=15,
                                        scalar2=0x64006400, op0=A.bitwise_and,
                                        op1=A.bitwise_or)
                nc.vector.tensor_scalar(out=z2[:osz], in0=z_t[:osz], scalar1=12,
                                        scalar2=0x000F0000,
                                        op0=A.logical_shift_left, op1=A.bitwise_and)
                nc.vector.tensor_tensor(out=z1[:osz], in0=z1[:osz], in1=z2[:osz],
                                        op=A.bitwise_or)
                zf = z1.bitcast(fp16)   # [128, GROUPS] = 1024 + z

                # per-group fp32 scalars: s32 = s ; zs32 = -(1024+z)*s
                s32 = deq.tile([128, GROUPS], f32, tag="s32", name="s32")
                nc.vector.tensor_copy(out=s32[:osz], in_=s_t[:osz])
                zs32 = deq.tile([128, GROUPS], f32, tag="zs32", name="zs32")
                nc.vector.tensor_tensor(out=zs32[:osz], in0=zf[:osz],
                                        in1=s_t[:osz], op=A.mult)
                nc.vector.tensor_scalar(out=zs32[:osz], in0=zs32[:osz],
                                        scalar1=-1.0, scalar2=None, op0=A.mult)

                # packed bytes -> interleaved fp16(1024+q) pairs
                t1 = deq.tile([128, K // 2], i32, tag="t1", name="t1")
                t2 = deq.tile([128, K // 2], i32, tag="t2", name="t2")
                nc.vector.tensor_scalar(out=t1[:osz], in0=q_t[:osz], scalar1=15,
                                        scalar2=0x64006400, op0=A.bitwise_and,
                                        op1=A.bitwise_or)
                nc.vector.tensor_scalar(out=t2[:osz], in0=q_t[:osz], scalar1=12,
                                        scalar2=0x000F0000,
                                        op0=A.logical_shift_left, op1=A.bitwise_and)
                nc.vector.tensor_tensor(out=t1[:osz], in0=t1[:osz], in1=t2[:osz],
                                        op=A.bitwise_or)
                vf = t1.bitcast(fp16)   # [128, K] = 1024 + q

                # dequant: w = vf * s + zs  (exactly (q - z) * s); odd groups
                # ride the otherwise-idle ACT engine (Identity activation
                # computes in*scale + bias with per-partition AP operands)
                AF = mybir.ActivationFunctionType
                wq = deq.tile([128, K], fp16, tag="wq", name="wq")
                for g in range(GROUPS):
                    src = vf[:osz, g * GROUP:(g + 1) * GROUP]
                    dst = wq[:osz, g * GROUP:(g + 1) * GROUP]
                    if g % 2 == 1:
                        nc.scalar.activation(dst, src, AF.Identity,
                                             bias=zs32[:osz, g:g + 1],
                                             scale=s32[:osz, g:g + 1])
                    else:
                        nc.vector.tensor_scalar(
                            out=dst, in0=src,
                            scalar1=s32[:osz, g:g + 1],
                            scalar2=zs32[:osz, g:g + 1],
                            op0=A.mult, op1=A.add)

                # transpose [osz, 128] blocks into wt[k, o]; batch 4 blocks
                # per PSUM bank and drain with one strided copy, alternating
                # DVE/ACT to halve the phase-A DVE load
                wt3 = wt.rearrange("p (kt oc) -> p kt oc", kt=KT)
                for kq in range(KT // 4):
                    pst = ppA.tile([128, 4, 128], fp16, tag="pst", name="pst")
                    for q in range(4):
                        kb = kq * 4 + q
                        nc.tensor.transpose(pst[:, q, :osz],
                                            wq[:osz, kb * 128:(kb + 1) * 128],
                                            ident[:osz, :osz])
                    dst = wt3[:, kq * 4:(kq + 1) * 4, o0:o0 + osz]
                    if kq % 2 == 0:
                        nc.scalar.copy(out=dst, in_=pst[:, :, :osz])
                    else:
                        nc.vector.tensor_copy(out=dst, in_=pst[:, :, :osz])

            # ---------------- Phase B: GEMM ----------------
            mb0_psts = {}

            def emit_mb0_chunk(j):
                # m-block 0, one column chunk across both m-subtiles;
                # interleaved with phase-A emission so the in-order PE
                # stream never waits on not-yet-dequantized wt columns
                xt = xts[0]
                if j == 0:
                    for part in range(0, KT, KT // 8):
                        nc.gpsimd.dma_start(
                            out=xt[:, part:part + KT // 8, :],
                            in_=xT_view[:, part:part + KT // 8, 0:XB])
                c0, c1 = CHUNKS[j]
                for sub in range(SUBS):
                    if j == 0:
                        mb0_psts[sub] = [
                            ppB.tile([128, 512], f32, tag=f"pp{jj}", name=f"pp{jj}")
                            for jj in range(len(CHUNKS))]
                    for kb in range(KT):
                        nc.tensor.matmul(
                            mb0_psts[sub][j][:, :c1 - c0],
                            lhsT=xt[:, kb, sub * 128:(sub + 1) * 128],
                            rhs=wt[:, kb * OC + c0: kb * OC + c1],
                            start=(kb == 0), stop=(kb == KT - 1))
                if j == len(CHUNKS) - 1:
                    for sub in range(SUBS):
                        ob = obs[sub % 2]
                        for jj, (d0, d1) in enumerate(CHUNKS):
                            nc.scalar.copy(out=ob[:, d0:d1],
                                           in_=mb0_psts[sub][jj][:, :d1 - d0])
                        nc.gpsimd.dma_start(out=out[sub * 128:(sub + 1) * 128, :],
                                            in_=ob)

            def emit_mblock(mb):
                xt = xts[mb % 2]
                # 8-way split spreads the block load across all SWDGE queues
                for part in range(0, KT, KT // 8):
                    nc.gpsimd.dma_start(
                        out=xt[:, part:part + KT // 8, :],
                        in_=xT_view[:, part:part + KT // 8, XB * mb:XB * (mb + 1)])
                for sub in range(SUBS):
                    mi = mb * SUBS + sub
                    psts = [ppB.tile([128, 512], f32, tag=f"pp{j}", name=f"pp{j}")
                            for j in range(len(CHUNKS))]
                    last = (mb == NB - 1 and sub == SUBS - 1)
                    if last:
                        # j-outer on the very last m-tile: chunk 0/1 drain
                        # (ACT copy + DMA) while chunk 2 still matmuls,
                        # shrinking the end-of-kernel serial tail
                        mm_order = [(j, kb) for j in range(len(CHUNKS))
                                    for kb in range(KT)]
                    else:
                        mm_order = [(j, kb) for kb in range(KT)
                                    for j in range(len(CHUNKS))]
                    ob = obs[mi % 2]
                    done = set()
                    for j, kb in mm_order:
                        c0, c1 = CHUNKS[j]
                        nc.tensor.matmul(
                            psts[j][:, :c1 - c0],
                            lhsT=xt[:, kb, sub * 128:(sub + 1) * 128],
                            rhs=wt[:, kb * OC + c0: kb * OC + c1],
                            start=(kb == 0), stop=(kb == KT - 1))
                        if last and kb == KT - 1:
                            nc.scalar.copy(out=ob[:, c0:c1], in_=psts[j][:, :c1 - c0])
                            nc.gpsimd.dma_start(out=out[mi * 128:(mi + 1) * 128, c0:c1],
                                                in_=ob[:, c0:c1])
                            done.add(j)
                    if not last:
                        for j, (c0, c1) in enumerate(CHUNKS):
                            nc.scalar.copy(out=ob[:, c0:c1], in_=psts[j][:, :c1 - c0])
                        nc.gpsimd.dma_start(out=out[mi * 128:(mi + 1) * 128, :], in_=ob)

            # interleaved emission: the PE stream alternates dequant
            # transposes with mb0 matmuls whose wt columns are ready
            for t in range(4):
                emit_otile(t)
            emit_mb0_chunk(0)
            for t in range(4, 8):
                emit_otile(t)
            emit_mb0_chunk(1)
            for t in range(8, OT):
                emit_otile(t)
            emit_mb0_chunk(2)
            for mb in range(1, NB):
                emit_mblock(mb)

    if not nc.is_finalized():
        nc.finalize()
    return nc


def kernel(x, qweight, scales, qzeros, group_size=128, **_unused):
    global LAST_RESULT
    from concourse.bass_utils import run_bass_kernel_spmd

    if "nc" not in _CACHE:
        _CACHE["nc"] = _build_bass()
    nc = _CACHE["nc"]

    x2d = np.asarray(x).reshape(M, K)
    xT = np.ascontiguousarray(x2d.T)   # [K, M] fp16
    qweight = np.asarray(qweight)
    scales = np.asarray(scales)
    qzeros = np.asarray(qzeros)

    in_maps = []
    for i in range(N_CORES):
        sl = slice(i * OC, (i + 1) * OC)
        in_maps.append({
            "xt_dram": xT,
            "qweight": np.ascontiguousarray(qweight[sl]),
            "scales": np.ascontiguousarray(scales[sl]),
            "qzeros": np.ascontiguousarray(qzeros[sl]),
        })

    res = run_bass_kernel_spmd(nc, in_maps, core_ids=list(range(N_CORES)),
                               **RUN_KWARGS)
    LAST_RESULT = res
    outs = [r["out"] for r in res.results]
    return np.concatenate(outs, axis=1).reshape(B, S, OUT_F).astype(np.float32)

